# revision 37
# baseline (speedup 1.0000x reference)
"""BFFN (linear-attention style gated FFN) Trainium2 Bass kernel, 8 NeuronCores.

Reference computation (all fp32, B=4, N=4096, D=E=1024):
    query = (x_real @ Wqr) * (x_imag @ Wqi)        # [b, n, e]
    key   = x_real @ Wk                             # [b, n, d]
    value = x_imag @ Wv                             # [b, n, e]
    kv    = einsum('bnd,bne->bde', key, value)      # [b, d, e]
    out   = einsum('bnd,bde->bne', query, kv)       # [b, n, e]

Algebraic restructure: kv = Wk^T @ (xr^T @ xi) @ Wv.  With S = xr^T @ xi
(the only sequence-length reduction), the kv path costs N*D*D + 2*D*D*E
instead of 3*N*D*E FLOPs, and S is computed from x in NATURAL layout.

Sharding: 8 cores = 4 batches x 2 sequence-halves.  Each pair AllReduces
its partial S (bf16, as two pipelined 1MB halves); each core then computes
its dk-HALF of kv (via a host-sliced wk input: even core gets wk cols
0:512, odd core 512:1024 -- the program stays uniform) and the pair
AllGathers the kv halves (concat lands in global dk order).

Schedule (v2), built to keep the PE fed end to end:
  stream phase: x streams f32 on both HWDGE queues -> DVE cast -> resident
    bf16.  Per-tile S quadrant Q00 (d 0:512, f 0:512) accumulates in 4 PSUM
    banks; per-chunk PE transposes + query chunks 0-1 interleave (weights
    stream on SWDGE from t=0, loaded in halves so query can start early).
  post-stream: S quadrants Q10 (replayed from resident x) -> bounce f-half0
    -> AllReduce0; Q01+Q11 -> AllReduce1 (both Shared-output, pipelined);
    query chunk 2; UT = S^T wk_half (f-half0 tiles gated on AR0 only);
    kv_own = UT^T wv; bounce -> AllGather; query chunk 3 covers the gather;
    out = qT.T @ kv with writes alternating both HWDGE queues.
All matmuls bf16 operands, fp32 PSUM accumulation.
"""
import numpy as np

import concourse.bass as bass
import concourse.mybir as mybir
import concourse.tile as tile
from concourse import bacc
from concourse.bass import ts, ds
from concourse.bass_utils import run_bass_kernel_spmd
from concourse.masks import make_identity

F32 = mybir.dt.float32
BF16 = mybir.dt.bfloat16

B, N, D, E = 4, 4096, 1024, 1024
N_CORES = 8
NL = N // 2          # 2048 rows (sequence) per core
P = 128
NT = NL // P         # 16 n-tiles
DT = D // P          # 8 d tiles
ET = E // P          # 8 e tiles
FD = 512             # matmul moving free dim / PSUM bank
NCH = NL // FD       # 4 n-chunks of 512
HK = 512             # dk half owned per core

REPLICA_GROUPS = [[0, 1], [2, 3], [4, 5], [6, 7]]


def build_bass():
    nc = bacc.Bacc("TRN2", target_bir_lowering=False, debug=False,
                   num_devices=N_CORES)

    xr = nc.dram_tensor("xr", [NL, D], F32, kind="ExternalInput").ap()
    xi = nc.dram_tensor("xi", [NL, D], F32, kind="ExternalInput").ap()
    wqr = nc.dram_tensor("wqr", [D, E], F32, kind="ExternalInput").ap()
    wqi = nc.dram_tensor("wqi", [D, E], F32, kind="ExternalInput").ap()
    wk = nc.dram_tensor("wk", [D, HK], F32, kind="ExternalInput").ap()
    wv = nc.dram_tensor("wv", [D, E], F32, kind="ExternalInput").ap()
    out = nc.dram_tensor("out", [NL, E], F32, kind="ExternalOutput").ap()

    def as_tiles(w):  # [1024, n] DRAM view -> [128, 8, n] partition-major
        return w.rearrange("(t p) n -> p t n", p=P)

    with tile.TileContext(nc) as tc:
        with (
            tc.tile_pool(name="big", bufs=3) as big_pool,      # x_nat/qt/s/kv
            tc.tile_pool(name="xs", bufs=3) as xs_pool,        # f32 staging
            tc.tile_pool(name="xtc", bufs=2) as xtc_pool,      # xT chunks
            tc.tile_pool(name="wp", bufs=1) as w_pool,
            tc.tile_pool(name="sst", bufs=2) as sst_pool,      # staging ring
            tc.tile_pool(name="prst", bufs=1) as prt_pool,
            tc.tile_pool(name="outst", bufs=2) as out_pool,
            tc.tile_pool(name="cst", bufs=1) as cst_pool,
            tc.tile_pool(name="ps", bufs=1, space="PSUM") as ps_pool,
            tc.tile_pool(name="dram", bufs=1, space="DRAM") as dram_pool,
        ):
            # DRAM bounce tensors; collective outputs Shared for fast path
            bnc_s_in = [dram_pool.tile([D, FD], BF16, tag=f"si{h}",
                                       name=f"bnc_s_in{h}") for h in range(2)]
            bnc_s_out = [dram_pool.tile([D, FD], BF16, tag=f"so{h}",
                                        name=f"bnc_s_out{h}") for h in range(2)]
            bnc_kv_in = [dram_pool.tile([HK, FD], BF16, tag=f"ki{h}",
                                        name=f"bnc_kv_in{h}") for h in range(2)]
            bnc_kv_out = [dram_pool.tile([D, FD], BF16, tag=f"ko{h}",
                                         name=f"bnc_kv_out{h}") for h in range(2)]

            ident = cst_pool.tile([P, P], BF16, tag="id", name="ident")
            make_identity(nc, ident)

            # ---- weight streams ----
            # wq e-half0 rides the HWDGE queues early (staged f32 pieces,
            # DVE cast) so query chunk 0 can start ~30us in; SWDGE is too
            # slow for that (~100GB/s: wq took 77us there in v2).  wq
            # e-half1 + wk + wv stream on SWDGE (needed much later).
            # wq streams on the HWDGE queues as FULL-ROW f32 pieces (4KB DMA
            # elements -- partial-row slices of the (t p) n view are strided
            # 2KB reads that stalled the x stream in v5), DVE-cast to bf16.
            # wk/wv ride SWDGE (slow, ~50GB/s, but needed only ~200us in).
            wqr_sb = w_pool.tile([P, DT, E], BF16, tag="wqr", name="wqr_sb")
            wqi_sb = w_pool.tile([P, DT, E], BF16, tag="wqi", name="wqi_sb")
            wk_sb = w_pool.tile([P, DT, HK], BF16, tag="wk", name="wk_sb")
            nc.gpsimd.dma_start(wk_sb[:], as_tiles(wk))
            wv_sb = w_pool.tile([P, DT, E], BF16, tag="wv", name="wv_sb")
            nc.gpsimd.dma_start(wv_sb[:], as_tiles(wv))

            def wq_piece(k):
                """Stage wq d-tiles 2k:2k+2 (all e) f32 on the HWDGE queues
                and cast into the bf16 weight tiles."""
                for w_dram, w_sb, eng, nm in ((wqr, wqr_sb, nc.sync, "r"),
                                              (wqi, wqi_sb, nc.scalar, "i")):
                    st = sst_pool.tile([P, 2, E], F32, tag="sst", bufs=2,
                                       name=f"wq_st{nm}")
                    eng.dma_start(st[:], as_tiles(w_dram)[:, 2 * k:2 * k + 2, :])
                    nc.vector.tensor_copy(w_sb[:, 2 * k:2 * k + 2, :], st[:])

            xr_nat = big_pool.tile([P, NT, D], BF16, tag="big", name="xr_nat")
            xi_nat = big_pool.tile([P, NT, D], BF16, tag="big", name="xi_nat")
            qt_sb = big_pool.tile([P, ET, NL], BF16, tag="big", name="qt_sb")

            def transpose_chunk(nch):
                """PE-transpose resident x chunk -> xt bf16 [P, DT, FD] x2."""
                res = {}
                for x_nat, kind in ((xr_nat, "r"), (xi_nat, "i")):
                    xt_c = xtc_pool.tile([P, DT, FD], BF16, tag="xtc",
                                         name=f"xt_{kind}")
                    for d in range(DT):
                        ps_t = ps_pool.tile([P, FD], BF16, tag="tp", bufs=1,
                                            name="ps_t")
                        for j in range(4):
                            nc.tensor.transpose(
                                ps_t[:, ts(j, P)],
                                x_nat[:, 4 * nch + j, ts(d, P)],
                                ident[:],
                            )
                        nc.vector.tensor_copy(xt_c[:, d, :], ps_t[:])
                    res[kind] = xt_c
                return res["r"], res["i"]

            def query_chunk(nch, xtr_c, xti_c, ets):
                for et in ets:
                    ps_r = ps_pool.tile([P, FD], F32, tag="q", bufs=3,
                                        name="ps_qr")
                    for d in range(DT):
                        nc.tensor.matmul(
                            ps_r[:], wqr_sb[:, d, ts(et, P)], xtr_c[:, d, :],
                            start=(d == 0), stop=(d == DT - 1),
                        )
                    prt = prt_pool.tile([P, FD], BF16, tag="prt", name="prt")
                    nc.vector.tensor_copy(prt[:], ps_r[:])
                    ps_i = ps_pool.tile([P, FD], F32, tag="q", bufs=3,
                                        name="ps_qi")
                    for d in range(DT):
                        nc.tensor.matmul(
                            ps_i[:], wqi_sb[:, d, ts(et, P)], xti_c[:, d, :],
                            start=(d == 0), stop=(d == DT - 1),
                        )
                    nc.vector.tensor_mul(
                        out=qt_sb[:, et, ts(nch, FD)], in0=prt[:], in1=ps_i[:],
                    )

            def s_quadrant(dlo, fh, first, last):
                """4 S accumulators [128,512] f32 for d-tiles dlo..dlo+3,
                f-half fh, accumulated over n-tiles first..last."""
                ps_s = [ps_pool.tile([P, FD], F32, tag="sacc", bufs=4,
                                     name=f"ps_s{dlo}_{fh}")
                        for _ in range(4)]
                for nt in range(first, last + 1):
                    for k in range(4):
                        nc.tensor.matmul(
                            ps_s[k][:], xr_nat[:, nt, ts(dlo + k, P)],
                            xi_nat[:, nt, ts(fh, FD)],
                            start=(nt == first), stop=(nt == last),
                        )
                return ps_s

            # ---- streaming phase ----
            chunk_xt = {}
            ps_q00 = None
            for nt in range(NT):
                # column-halves: the S quadrant needs only h0 of both
                # tensors, so it starts on half-arrived tiles.
                xs_r = xs_pool.tile([P, D], F32, tag="xs", name="xs_r")
                xs_i = xs_pool.tile([P, D], F32, tag="xs", name="xs_i")
                nc.sync.dma_start(xs_r[:, :FD], xr[ts(nt, P), :FD])
                nc.scalar.dma_start(xs_i[:, :FD], xi[ts(nt, P), :FD])
                nc.vector.tensor_copy(xr_nat[:, nt, :FD], xs_r[:, :FD])
                nc.vector.tensor_copy(xi_nat[:, nt, :FD], xs_i[:, :FD])
                nc.sync.dma_start(xs_r[:, FD:], xr[ts(nt, P), FD:])
                nc.scalar.dma_start(xs_i[:, FD:], xi[ts(nt, P), FD:])
                if nt == 0:
                    ps_q00 = [ps_pool.tile([P, FD], F32, tag="sacc", bufs=4,
                                           name="ps_q00")
                              for _ in range(4)]
                for k in range(4):
                    nc.tensor.matmul(
                        ps_q00[k][:], xr_nat[:, nt, ts(k, P)],
                        xi_nat[:, nt, :FD],
                        start=(nt == 0), stop=(nt == NT - 1),
                    )
                nc.vector.tensor_copy(xr_nat[:, nt, FD:], xs_r[:, FD:])
                nc.vector.tensor_copy(xi_nat[:, nt, FD:], xs_i[:, FD:])
                # query chunk c is emitted BEFORE transpose chunk c+1 so the
                # xtc slot rotation (bufs=2) never waits on a later reader.
                if nt in (1, 3, 5, 7):
                    wq_piece(nt // 2)
                if nt == 3:
                    chunk_xt[0] = transpose_chunk(0)
                if nt == 7:
                    query_chunk(0, *chunk_xt[0], range(ET))
                    chunk_xt[1] = transpose_chunk(1)
                if nt == 9:
                    query_chunk(1, *chunk_xt[1], range(ET))
                if nt == 11:
                    chunk_xt[2] = transpose_chunk(2)

            # ---- S replay quadrants + pipelined AllReduce halves ----
            s_st0 = sst_pool.tile([P, DT, FD], BF16, tag="sst", name="s_st0")
            for k in range(4):
                nc.vector.tensor_copy(s_st0[:, k, :], ps_q00[k][:])
            ps_q10 = s_quadrant(4, 0, 0, NT - 1)
            for k in range(4):
                nc.vector.tensor_copy(s_st0[:, 4 + k, :], ps_q10[k][:])
            nc.sync.dma_start(as_tiles(bnc_s_in[0])[:], s_st0[:])
            nc.gpsimd.collective_compute(
                "AllReduce", mybir.AluOpType.add,
                replica_groups=REPLICA_GROUPS,
                ins=[bnc_s_in[0].opt()], outs=[bnc_s_out[0].opt()],
            )

            s_st1 = sst_pool.tile([P, DT, FD], BF16, tag="sst", name="s_st1")
            ps_q01 = s_quadrant(0, 1, 0, NT - 1)
            for k in range(4):
                nc.vector.tensor_copy(s_st1[:, k, :], ps_q01[k][:])
            ps_q11 = s_quadrant(4, 1, 0, NT - 1)
            for k in range(4):
                nc.vector.tensor_copy(s_st1[:, 4 + k, :], ps_q11[k][:])
            nc.sync.dma_start(as_tiles(bnc_s_in[1])[:], s_st1[:])
            nc.gpsimd.collective_compute(
                "AllReduce", mybir.AluOpType.add,
                replica_groups=REPLICA_GROUPS,
                ins=[bnc_s_in[1].opt()], outs=[bnc_s_out[1].opt()],
            )

            # ---- query chunk 2 (covers AR0/AR1 latency) + late c3 transposes
            query_chunk(2, *chunk_xt[2], range(ET))
            chunk_xt[3] = transpose_chunk(3)

            # reduced S: recycle xr_nat's slot (dead after Q11 + c3 transp)
            s_sb = big_pool.tile([P, DT, D], BF16, tag="big", name="s_sb")
            nc.scalar.dma_start(s_sb[:, :, :FD], as_tiles(bnc_s_out[0]))
            nc.scalar.dma_start(s_sb[:, :, FD:], as_tiles(bnc_s_out[1]))

            # ---- UT = S^T wk_half: [f 1024, dk-own 512] ----
            ut_sb = sst_pool.tile([P, DT, HK], BF16, tag="sst", name="ut_sb")
            for dpt in range(DT):      # f-tile of UT's partition dim
                ps_u = ps_pool.tile([P, HK], F32, tag="q", bufs=3, name="ps_u")
                for d in range(DT):
                    nc.tensor.matmul(
                        ps_u[:], s_sb[:, d, ts(dpt, P)], wk_sb[:, d, :],
                        start=(d == 0), stop=(d == DT - 1),
                    )
                nc.vector.tensor_copy(ut_sb[:, dpt, :], ps_u[:])

            # ---- kv_own = UT^T wv: [dk-own 512, e 1024], e-half pipelined
            # AllGathers so the out phase's eh0/eh1 accumulations gate
            # independently (a single AG left a ~12us exposed tail).
            kv_st = sst_pool.tile([P, 4, E], BF16, tag="sst", name="kv_st")
            for eh in range(2):
                for dkt in range(4):   # local dk tile
                    ps_k = ps_pool.tile([P, FD], F32, tag="q", bufs=3,
                                        name="ps_k")
                    for dp in range(DT):
                        nc.tensor.matmul(
                            ps_k[:], ut_sb[:, dp, ts(dkt, P)],
                            wv_sb[:, dp, ts(eh, FD)],
                            start=(dp == 0), stop=(dp == DT - 1),
                        )
                    nc.vector.tensor_copy(kv_st[:, dkt, ts(eh, FD)], ps_k[:])
                nc.sync.dma_start(
                    bnc_kv_in[eh].rearrange("(t p) n -> p t n", p=P)[:],
                    kv_st[:, :, ts(eh, FD)])
                nc.gpsimd.collective_compute(
                    "AllGather", mybir.AluOpType.bypass,
                    replica_groups=REPLICA_GROUPS,
                    ins=[bnc_kv_in[eh].opt()], outs=[bnc_kv_out[eh].opt()],
                )

            # ---- query chunk 3 (covers the kv AllGathers) ----
            query_chunk(3, *chunk_xt[3], range(ET))

            # full kv in global dk order: recycle xi_nat's slot
            kv_sb = big_pool.tile([P, DT, E], BF16, tag="big", name="kv_sb")
            for eh in range(2):
                nc.scalar.dma_start(kv_sb[:, :, ts(eh, FD)],
                                    as_tiles(bnc_kv_out[eh]))

            # ---- out = queryT.T @ kv ----
            for nt in range(NT):
                ps_o = [ps_pool.tile([P, FD], F32, tag="q", bufs=3,
                                     name="ps_o") for _ in range(2)]
                for et in range(ET):
                    lhsT = qt_sb[:, et, ts(nt, P)]
                    for eh in range(2):
                        nc.tensor.matmul(
                            ps_o[eh][:], lhsT, kv_sb[:, et, ts(eh, FD)],
                            start=(et == 0), stop=(et == ET - 1),
                        )
                for eh in range(2):
                    o_st = out_pool.tile([P, FD], F32, tag="ost", name="o_st")
                    nc.vector.tensor_copy(o_st[:], ps_o[eh][:])
                    eng = (nc.sync, nc.scalar, nc.gpsimd)[(2 * nt + eh) % 3]
                    eng.dma_start(out[ts(nt, P), ts(eh, FD)], o_st[:])

    nc.compile()
    return nc


def make_in_maps(x_real, x_imag, w_query_real, w_query_imag, w_key, w_value):
    ws = {
        "wqr": np.ascontiguousarray(w_query_real, dtype=np.float32),
        "wqi": np.ascontiguousarray(w_query_imag, dtype=np.float32),
        "wv": np.ascontiguousarray(w_value, dtype=np.float32),
    }
    wk_halves = [np.ascontiguousarray(w_key[:, h * HK:(h + 1) * HK],
                                      dtype=np.float32) for h in range(2)]
    in_maps = []
    for c in range(N_CORES):
        b, h = divmod(c, 2)
        sl = slice(h * NL, (h + 1) * NL)
        in_maps.append({
            "xr": np.ascontiguousarray(x_real[b, sl], dtype=np.float32),
            "xi": np.ascontiguousarray(x_imag[b, sl], dtype=np.float32),
            "wk": wk_halves[h],
            **ws,
        })
    return in_maps


def gather_out(results):
    out = np.empty((B, N, E), np.float32)
    for c in range(N_CORES):
        b, h = divmod(c, 2)
        out[b, h * NL:(h + 1) * NL] = results[c]["out"]
    return out


def kernel(x_real, x_imag, w_query_real, w_query_imag, w_key, w_value):
    nc = build_bass()
    in_maps = make_in_maps(x_real, x_imag, w_query_real, w_query_imag,
                           w_key, w_value)
    res = run_bass_kernel_spmd(nc, in_maps, core_ids=list(range(N_CORES)))
    return gather_out(res.results)


if __name__ == "__main__":
    rng = np.random.default_rng(0)
    args = dict(
        x_real=rng.standard_normal((B, N, D), dtype=np.float32),
        x_imag=rng.standard_normal((B, N, D), dtype=np.float32),
        w_query_real=(rng.standard_normal((D, E), dtype=np.float32) / D),
        w_query_imag=(rng.standard_normal((D, E), dtype=np.float32) / D),
        w_key=(rng.standard_normal((D, E), dtype=np.float32) / D),
        w_value=(rng.standard_normal((D, E), dtype=np.float32) / D),
    )
    got = kernel(**args)
    q = np.einsum("bnd,de->bne", args["x_real"], args["w_query_real"]) * \
        np.einsum("bnd,de->bne", args["x_imag"], args["w_query_imag"])
    k = np.einsum("bnd,de->bne", args["x_real"], args["w_key"])
    v = np.einsum("bnd,de->bne", args["x_imag"], args["w_value"])
    kv = np.einsum("bnd,bne->bde", k, v)
    want = np.einsum("bnd,bde->bne", q, kv)
    denom = np.abs(want).max()
    print("max abs err:", np.abs(got - want).max())
    print("rel err:", np.abs(got - want).max() / denom)
